# revision 45
# baseline (speedup 1.0000x reference)
"""Multi-head attention (B=8, N=1024, EMB=768, H=12, D=64) on 8 trn2 cores.

Strategy: data-parallel over batch (1 batch element per core, no collectives).

v5 (PE-dense schedule rework over v4.5):
  - PSUM repartition: dedicated S pool (2x[128,1024]) so S-psum rotation
    never waits on qkT-CAST drains; qkT psums in their own [128,512]x2
    pool (reused by y output-projection halves in the tail); AV pool
    unchanged (2x[128,512]).
  - DMA re-route: xT chunks 0-2 + bias on the scalar queue (then the
    scalar engine is exp-only), xT 3-5 + strips + wv batched on sync.
    A dummy exp right after memset preloads the ACT exp table during
    engine bring-up, so the first real exp issues without the ~2.7us
    table-load stall.
  - qkT prefetch spread across both halves of each pair (always-ready PE
    filler at half boundaries); pair-5 attention interleaves the y
    output projection (k<=4 partials + nn0 finishers) so the tail after
    the last normalize is only the nn1 k=5 matmuls + ACT copies.
  - y bias folded into the ACT drain copy (activation Copy with
    per-partition bias) - no more K=1 bias matmuls, no DVE y copies.
  - y computed transposed as before; output DMAs stream per [128,512]
    half as soon as each is finalized.
"""

import numpy as np
import ml_dtypes
from contextlib import ExitStack

import concourse.bass as bass
import concourse.bacc as bacc
import concourse.tile as tile
from concourse import mybir
from concourse.bass_utils import run_bass_kernel_spmd

B, N, EMB = 8, 1024, 768
H, D = 12, 64
ATT = H * D          # 768
P = 128
NT = N // P          # 8 token chunks
EC = EMB // P        # 6 emb chunks
NP = H // 2          # 6 head pairs
FP = mybir.dt.float32
BF = mybir.dt.bfloat16
SCALE = 1.0 / float(np.sqrt(D))
HW = 512             # query half width

N_CORES = 8


def _emit_kernel(tc, xT_d, wqkv_d, wout_d, bT_d, y_d):
    nc = tc.nc
    with ExitStack() as ctx:
        const = ctx.enter_context(tc.tile_pool(name="const", bufs=1))
        warm_sb = const.tile([P, HW], BF)
        nc.gpsimd.memset(warm_sb, 0.03125)
        dummy = const.tile([P, 8], FP)
        nc.gpsimd.memset(dummy, 0.125)
        dummy_o = const.tile([P, 8], FP)
        # preload the exp table set while the DMA queues are still priming
        nc.scalar.activation(dummy_o, dummy,
                             mybir.ActivationFunctionType.Exp)
        bT_sb = const.tile([P, EC], FP)

        outT_pool = ctx.enter_context(tc.tile_pool(name="outT", bufs=1,
                                                   side="right"))
        outT = [
            outT_pool.tile([P, N], BF, tag=f"outT{m}", name=f"outT{m}")
            for m in range(NP)
        ]
        wout_pool = ctx.enter_context(tc.tile_pool(name="wout", bufs=1,
                                                   side="right"))
        vaug_pool = ctx.enter_context(tc.tile_pool(name="vaugp", bufs=1,
                                                   side="right"))
        y_pool = ctx.enter_context(tc.tile_pool(name="y", bufs=1))

        with tc.tile_pool(name="weights", bufs=1) as wpool, \
             tc.tile_pool(name="att", bufs=1) as att, \
             tc.tile_pool(name="wvp", bufs=1) as wv_pool:

            strips = {}

            def emit_strip_dmas(p, eng=None):
                """Per-pair w_q/w_k column strips [128, 2, EC, 128], one
                batched DMA per q/k."""
                e = eng or nc.sync
                st = wpool.tile([P, 2, EC, P], BF, tag="strip", bufs=3,
                                name=f"st{p}")
                for qk, col0 in ((0, p * P), (1, ATT + p * P)):
                    e.dma_start(
                        out=st[:, qk, :, :],
                        in_=wqkv_d[:, col0:col0 + P]
                        .rearrange("(k r) c -> r k c", r=P))
                strips[p] = st

            spool = ctx.enter_context(tc.tile_pool(name="ps_s", bufs=1,
                                                   space="PSUM"))
            qpool = ctx.enter_context(tc.tile_pool(name="ps_q", bufs=1,
                                                   space="PSUM"))
            avpool = ctx.enter_context(tc.tile_pool(name="ps_av", bufs=1,
                                                    space="PSUM"))

            # ---- PE warm-up: full-K matmuls engage the HAM un-throttle
            # while the startup DMAs land.
            def emit_warm(n):
                for _ in range(n):
                    ps_warm = spool.tile([P, N], FP, tag="s", bufs=2,
                                         name="warm")
                    nc.tensor.matmul(ps_warm[:, 0:HW], warm_sb[:, 0:P],
                                     warm_sb, start=True, stop=True)

            emit_warm(12)

            # ---- startup DMAs, priority order ------------------------
            # sync: pair-0 strips, xT chunks 3-5, wv, pair-1+ strips
            # scalar: xT chunks 0-2 + bias (then the ACT queue is exps only)
            emit_strip_dmas(0)
            xt6 = wpool.tile([P, EC, N], BF, tag="xT6", name="xT6")
            for k in range(3):
                nc.scalar.dma_start(out=xt6[:, k, :],
                                    in_=xT_d[k * P:(k + 1) * P, :])
            nc.scalar.dma_start(out=bT_sb, in_=bT_d[:])
            nc.sync.dma_start(
                out=xt6[:, 3:6, :],
                in_=xT_d[3 * P:6 * P, :].rearrange("(k r) c -> r k c", r=P))
            xT = [xt6[:, k, :] for k in range(EC)]
            wv6 = wv_pool.tile([P, EC, EMB], BF, tag="wv6", name="wv6")
            nc.sync.dma_start(
                out=wv6, in_=wqkv_d[:, 2 * ATT:]
                .rearrange("(k r) c -> r k c", r=P))
            wv_sb = [wv6[:, k, :] for k in range(EC)]
            emit_strip_dmas(1)
            emit_strip_dmas(2)

            # ---- helper emitters ---------------------------------------
            class QKTEmitter:
                """Emits a pair's 24 qkT matmuls a few per call: steady
                always-ready PE filler between the es-gated AV/S work."""

                def __init__(self, p):
                    self.p = p
                    self.jobs = [(qk, nn, k)
                                 for qk in range(2)
                                 for nn in range(2)
                                 for k in range(EC)]
                    self.idx = 0
                    self.psq = None
                    self.tiles = [None, None]

                def done(self):
                    return self.idx >= len(self.jobs)

                def step(self, n=3):
                    while n > 0 and not self.done():
                        qk, nn, k = self.jobs[self.idx]
                        which = "qk"[qk]
                        if self.tiles[qk] is None:
                            self.tiles[qk] = wpool.tile(
                                [P, N], BF, tag=f"{which}Tp", bufs=3,
                                name=f"{which}T{self.p}")
                        if self.psq is None:
                            self.psq = qpool.tile(
                                [P, HW], FP, tag="q", bufs=2,
                                name=f"ps{which}{self.p}_{nn}")
                        nc.tensor.matmul(
                            self.psq,
                            strips[self.p][:, qk, k, :],
                            xT[k][:, nn * HW:(nn + 1) * HW],
                            start=(k == 0),
                            stop=(k == EC - 1),
                        )
                        self.idx += 1
                        n -= 1
                        if k == EC - 1:
                            nc.vector.tensor_copy(
                                self.tiles[qk][:, nn * HW:(nn + 1) * HW],
                                self.psq)
                            self.psq = None
                    if self.done():
                        qkt[self.p] = self.tiles
                    return self.done()

            def emit_qkT(p):
                em = QKTEmitter(p)
                em.step(len(em.jobs))
                return qkt[p]

            def emit_v(t):
                va = vaug_pool.tile([P, H, P], BF, tag=f"vaug{t}",
                                    name=f"vaug{t}")
                nc.gpsimd.memset(va[:, :, D:P], 1.0)
                for (n0, n1) in ((0, 512), (512, 768)):
                    psv = avpool.tile([P, HW], FP, tag="av", bufs=2,
                                      name=f"psv{t}_{n0}")
                    w = n1 - n0
                    for k in range(EC):
                        nc.tensor.matmul(
                            psv[:, 0:w],
                            xT[k][:, t * P:(t + 1) * P],
                            wv_sb[k][:, n0:n1],
                            start=(k == 0),
                            stop=(k == EC - 1),
                        )
                    nc.vector.tensor_copy(
                        va[:, n0 // D:n1 // D, 0:D],
                        psv[:, 0:w].rearrange("p (h d) -> p h d", d=D),
                    )
                return va

            qkt = {}
            es_tiles = {}
            s_cursor = [0]

            def emit_S_unit(p, c, half):
                """S^T for key-chunk c, query-half `half`, BOTH heads in one
                PSUM tile (h0 -> cols 0:512 via row group 0, h1 -> cols
                512:1024 via row group 64). Sharing one tile means both
                matmuls' rotation dependency resolves together, so the
                row-group 2x packing engages every time; one exp covers
                both heads (per-exp instruction overhead is ~125ns, so
                fewer, larger exps win)."""
                qT, kT = qkt[p]
                t = spool.tile([P, N], FP, tag="s", bufs=2,
                               name=f"s{p}_{c}_{half}")
                for i, base in ((0, 0), (1, 64)):
                    nc.tensor.matmul(
                        t[:, i * HW:(i + 1) * HW],
                        kT[base:base + D, c * P:(c + 1) * P],
                        qT[base:base + D, half * HW:(half + 1) * HW],
                        start=True,
                        stop=True,
                    )
                es = att.tile([P, N], BF, tag="expS", bufs=28,
                              name=f"es{p}_{c}_{half}")
                nc.scalar.activation(
                    es, t, mybir.ActivationFunctionType.Exp, scale=SCALE)
                return es

            def pump_S(limit):
                while s_cursor[0] < min(limit, NP * NT * 2):
                    u = s_cursor[0]
                    p_, w = divmod(u, 2 * NT)
                    half_, c_ = divmod(w, NT)
                    if p_ not in qkt:
                        break
                    es_tiles[(p_, c_, half_)] = emit_S_unit(p_, c_, half_)
                    s_cursor[0] += 1

            def emit_normalize(p, i, half, av_t):
                # Plain-op Newton reciprocal: seed z0 = bitcast(~x)*c0 = -y0,
                # one NR step z1 = (x*z0 + 2)*z0 = -y1, final multiply folds
                # the sign: outT = (raw * -1) * z1 = raw * y1.
                # (custom DVE ops like reciprocal_approx_fast run in CoreSim
                # but produce garbage on this runtime - plain ops only)
                x = av_t[D:2 * D, :]               # [64,512] denom (repl)
                nx = att.tile([D, HW], FP, tag="nrm", bufs=4,
                              name=f"nx{p}_{i}_{half}")
                nc.vector.tensor_scalar(
                    out=nx.bitcast(mybir.dt.int32),
                    in0=x.bitcast(mybir.dt.int32),
                    scalar1=-1, scalar2=None,
                    op0=mybir.AluOpType.bitwise_xor)
                z0 = att.tile([D, HW], FP, tag="nrm", bufs=4,
                              name=f"z0{p}_{i}_{half}")
                nc.vector.tensor_scalar_mul(z0, nx, 0.23549792)
                pr = att.tile([D, HW], FP, tag="nrm", bufs=4,
                              name=f"pr{p}_{i}_{half}")
                nc.vector.tensor_mul(pr, x, z0)
                z1 = att.tile([D, HW], FP, tag="nrm", bufs=4,
                              name=f"z1{p}_{i}_{half}")
                nc.vector.scalar_tensor_tensor(
                    out=z1, in0=pr, scalar=-2.0, in1=z0,
                    op0=mybir.AluOpType.subtract,
                    op1=mybir.AluOpType.mult)
                nc.vector.scalar_tensor_tensor(
                    out=outT[p][i * D:(i + 1) * D,
                                half * HW:(half + 1) * HW],
                    in0=av_t[0:D, :], scalar=-1.0, in1=z1,
                    op0=mybir.AluOpType.mult,
                    op1=mybir.AluOpType.mult)

            # ---- y output projection helpers ---------------------------
            y_live = {}

            def emit_y(e, nn, ks, finish=False, drain="vector", psum=None):
                t = y_live.get((e, nn))
                if t is None:
                    if psum is None:
                        psum = qpool.tile([P, HW], FP, tag="q", bufs=2,
                                          name=f"psy{e}_{nn}")
                    t = psum
                    y_live[(e, nn)] = t
                for k in ks:
                    nc.tensor.matmul(
                        t,
                        wout_sb[k][:, e * P:(e + 1) * P],
                        outT[k][:, nn * HW:(nn + 1) * HW],
                        start=(k == 0),
                        stop=(finish and k == NP - 1),
                    )
                if finish:
                    y_sb = y_pool.tile([P, HW], FP, tag="y", bufs=4,
                                       name=f"ysb{e}_{nn}")
                    if drain == "vector":
                        # DVE is idle here; +bias via per-partition scalar
                        nc.vector.tensor_scalar_add(
                            y_sb, t, bT_sb[:, e:e + 1])
                    else:
                        # ACT is idle post-exp; Identity allows AP bias
                        nc.scalar.activation(
                            y_sb, t, mybir.ActivationFunctionType.Identity,
                            bias=bT_sb[:, e:e + 1])
                    dma_eng = nc.sync if e % 2 == 0 else nc.scalar
                    dma_eng.dma_start(
                        out=y_d[e * P:(e + 1) * P, nn * HW:(nn + 1) * HW],
                        in_=y_sb)
                    del y_live[(e, nn)]

            # ---- prologue: qkT(0) (warm-MM filler absorbs the DMA
            # arrival jitter), pump, qkT(1), then S+v interleave -------
            qkt0 = QKTEmitter(0)
            while not qkt0.done():
                qkt0.step(6)
                emit_warm(3)
            pump_S(2)
            qkt1 = QKTEmitter(1)
            for _ in range(3):
                qkt1.step(8)
                pump_S(s_cursor[0] + 1)
            vaug = []
            for c in range(NT):
                pump_S(2 * (c + 1) + 3)
                vaug.append(emit_v(c))
                if c == 0:
                    emit_strip_dmas(3)
                if c == 4:
                    emit_strip_dmas(4)

            # w_out load rides the sync queue here (arrives mid-attention)
            wo6 = wout_pool.tile([P, EC, EMB], BF, tag="wout6", name="wout6")
            nc.sync.dma_start(
                out=wo6, in_=wout_d.rearrange("(k r) c -> r k c", r=P))
            wout_sb = [wo6[:, k, :] for k in range(EC)]

            # ---- main pair loop (AV per query-half) --------------------
            for p in range(NP):
                qkt_next = QKTEmitter(p + 2) if p + 2 < NP else None
                for half in range(2):
                    av_ts = [
                        avpool.tile([P, HW], FP, tag="av", bufs=2,
                                    name=f"av{p}_{i}_{half}")
                        for i in range(2)
                    ]
                    for c in range(NT):
                        # The PE queue is strict FIFO: put stall-prone S
                        # matmuls AFTER always-ready AV/qkT work, except at
                        # the half boundary (c==0) where AV itself waits on
                        # the av-psum rotation and S is the filler.
                        if c == 0:
                            pump_S((p * 2 + half) * NT + c + 2 * NT + 6)
                        for i in range(2):
                            nc.tensor.matmul(
                                av_ts[i],
                                vaug[c][:, 2 * p + i, :],
                                es_tiles[(p, c, half)][:,
                                                       i * HW:(i + 1) * HW],
                                start=(c == 0),
                                stop=(c == NT - 1),
                            )
                        if qkt_next is not None:
                            qkt_next.step(2 if half == 0 else 1)
                        if half == 0 and c == 4 and p == 0 and NP > 5:
                            emit_strip_dmas(5)
                        if p == NP - 1:
                            # weave the output projection into pair 5:
                            # k<=4 partials are dependency-free filler; the
                            # k=5 finishers wait only normalize(5, half=0)
                            # which completes early in half 1.
                            if half == 0 and c == 2:
                                emit_y(0, 0, range(NP - 1))
                            if half == 0 and c == 5:
                                emit_y(1, 0, range(NP - 1))
                            if half == 1 and c == 4:
                                emit_y(0, 0, [NP - 1], finish=True)
                            if half == 1 and c == 5:
                                emit_y(1, 0, [NP - 1], finish=True)
                            if half == 1 and c == 6:
                                emit_y(2, 0, range(NP), finish=True,
                                       drain="scalar")
                            if half == 1 and c == 7:
                                emit_y(3, 0, range(NP), finish=True,
                                       drain="scalar")
                        if c > 0:
                            pump_S((p * 2 + half) * NT + c + 2 * NT + 6)
                    if qkt_next is not None and half == 0:
                        qkt_next.step(4)
                    emit_normalize(p, 0, half, av_ts[0])
                    emit_normalize(p, 1, half, av_ts[1])
                    # keep the HAM activity monitor fed across the
                    # normalize-chain stall (LDWEIGHTS needs no PSUM bank)
                    for _ in range(3):
                        nc.tensor.ldweights(weights=warm_sb[:, 0:P])

        # ---- tail: the S-pool tiles are drained by now; use them as
        # extra y accumulators (one [128,N] tile hosts two y halves) so
        # all remaining output-projection matmuls run back-to-back.
        # nn=0 work (only needs normalize(5,0), long done) goes first
        # and covers the normalize(5,1) DVE chains; nn=1 k=5 last.
        sA = spool.tile([P, N], FP, tag="s", bufs=2, name="ytail_a")
        emit_y(4, 0, range(NP), finish=True, drain="scalar",
               psum=sA[:, 0:HW])
        emit_y(5, 0, range(NP), finish=True, drain="scalar",
               psum=sA[:, HW:N])
        sB = spool.tile([P, N], FP, tag="s", bufs=2, name="ytail_b")
        emit_y(0, 1, range(NP - 1), psum=sB[:, 0:HW])
        emit_y(1, 1, range(NP - 1), psum=sB[:, HW:N])
        sC = spool.tile([P, N], FP, tag="s", bufs=2, name="ytail_c")
        emit_y(4, 1, range(NP - 1), psum=sC[:, 0:HW])
        emit_y(5, 1, range(NP - 1), psum=sC[:, HW:N])
        emit_y(2, 1, range(NP - 1))
        emit_y(3, 1, range(NP - 1))
        # k=5 finishers: drains alternate ACT/DVE so the last few pipeline
        emit_y(0, 1, [NP - 1], finish=True, drain="scalar")
        emit_y(1, 1, [NP - 1], finish=True, drain="vector")
        emit_y(4, 1, [NP - 1], finish=True, drain="scalar")
        emit_y(5, 1, [NP - 1], finish=True, drain="vector")
        emit_y(2, 1, [NP - 1], finish=True, drain="scalar")
        emit_y(3, 1, [NP - 1], finish=True, drain="vector")


_NC_CACHE = None


def _build_nc(reps=1):
    global _NC_CACHE
    if reps == 1 and _NC_CACHE is not None:
        return _NC_CACHE
    nc = bacc.Bacc("TRN2", target_bir_lowering=False, debug=False,
                   num_devices=N_CORES)
    xT_d = nc.declare_dram_parameter("xT", [EMB, N], BF, isOutput=False)
    wqkv_d = nc.declare_dram_parameter("w_qkv", [EMB, 3 * ATT], BF, isOutput=False)
    wout_d = nc.declare_dram_parameter("w_out", [ATT, EMB], BF, isOutput=False)
    bT_d = nc.declare_dram_parameter("bT", [P, EC], FP, isOutput=False)
    y_d = nc.declare_dram_parameter("y", [EMB, N], FP, isOutput=True)
    with tile.TileContext(nc) as tc:
        for _ in range(reps):
            _emit_kernel(tc, xT_d, wqkv_d, wout_d, bT_d, y_d)
    nc.compile()
    if reps == 1:
        _NC_CACHE = nc
    return nc


def make_in_maps(x, w_qkv, w_out, b_out):
    BFnp = ml_dtypes.bfloat16
    x = np.asarray(x, dtype=np.float32)
    w_qkv = np.asarray(w_qkv, dtype=np.float32).astype(BFnp)
    w_out = np.asarray(w_out, dtype=np.float32).astype(BFnp)
    bT = np.ascontiguousarray(
        np.asarray(b_out, dtype=np.float32).reshape(EC, P).T)
    assert x.shape == (B, N, EMB)
    xT = [np.ascontiguousarray(x[i].T).astype(BFnp) for i in range(B)]
    return [
        {"xT": xT[i], "w_qkv": w_qkv, "w_out": w_out, "bT": bT}
        for i in range(N_CORES)
    ]


def postprocess_core_output(y):
    return np.ascontiguousarray(np.asarray(y).T)


def run_sharded(x, w_qkv, w_out, b_out, **run_kwargs):
    """Shard over batch, run on 8 cores, gather. Returns (out, BassKernelResults)."""
    nc = _build_nc()
    in_maps = make_in_maps(x, w_qkv, w_out, b_out)
    res = run_bass_kernel_spmd(nc, in_maps, core_ids=list(range(N_CORES)),
                               **run_kwargs)
    out = np.stack(
        [postprocess_core_output(res.results[i]["y"]) for i in range(N_CORES)],
        axis=0)
    return out, res


def kernel(x, w_qkv, w_out, b_out):
    out, _ = run_sharded(x, w_qkv, w_out, b_out)
    return out


# revision 48
# speedup vs baseline: 1.0189x; 1.0189x over previous
"""Multi-head attention (B=8, N=1024, EMB=768, H=12, D=64) on 8 trn2 cores.

Strategy: data-parallel over batch (1 batch element per core, no collectives).

v5 (PE-dense schedule rework over v4.5):
  - PSUM repartition: dedicated S pool (2x[128,1024]) so S-psum rotation
    never waits on qkT-CAST drains; qkT psums in their own [128,512]x2
    pool (reused by y output-projection halves in the tail); AV pool
    unchanged (2x[128,512]).
  - DMA re-route: xT chunks 0-2 + bias on the scalar queue (then the
    scalar engine is exp-only), xT 3-5 + strips + wv batched on sync.
    A dummy exp right after memset preloads the ACT exp table during
    engine bring-up, so the first real exp issues without the ~2.7us
    table-load stall.
  - qkT prefetch spread across both halves of each pair (always-ready PE
    filler at half boundaries); pair-5 attention interleaves the y
    output projection (k<=4 partials + nn0 finishers) so the tail after
    the last normalize is only the nn1 k=5 matmuls + ACT copies.
  - y bias folded into the ACT drain copy (activation Copy with
    per-partition bias) - no more K=1 bias matmuls, no DVE y copies.
  - y computed transposed as before; output DMAs stream per [128,512]
    half as soon as each is finalized.
"""

import numpy as np
import ml_dtypes
from contextlib import ExitStack

import concourse.bass as bass
import concourse.bacc as bacc
import concourse.tile as tile
from concourse import mybir
from concourse.bass_utils import run_bass_kernel_spmd

B, N, EMB = 8, 1024, 768
H, D = 12, 64
ATT = H * D          # 768
P = 128
NT = N // P          # 8 token chunks
EC = EMB // P        # 6 emb chunks
NP = H // 2          # 6 head pairs
FP = mybir.dt.float32
BF = mybir.dt.bfloat16
SCALE = 1.0 / float(np.sqrt(D))
HW = 512             # query half width

N_CORES = 8


def _emit_kernel(tc, xT_d, wqkv_d, wout_d, bT_d, y_d):
    nc = tc.nc
    with ExitStack() as ctx:
        const = ctx.enter_context(tc.tile_pool(name="const", bufs=1))
        warm_sb = const.tile([P, HW], BF)
        nc.gpsimd.memset(warm_sb, 0.03125)
        dummy = const.tile([P, 8], FP)
        nc.gpsimd.memset(dummy, 0.125)
        dummy_o = const.tile([P, 8], FP)
        # preload the exp table set while the DMA queues are still priming
        nc.scalar.activation(dummy_o, dummy,
                             mybir.ActivationFunctionType.Exp)
        bT_sb = const.tile([P, EC], FP)

        outT_pool = ctx.enter_context(tc.tile_pool(name="outT", bufs=1,
                                                   side="right"))
        outT = [
            outT_pool.tile([P, N], BF, tag=f"outT{m}", name=f"outT{m}")
            for m in range(NP)
        ]
        wout_pool = ctx.enter_context(tc.tile_pool(name="wout", bufs=1,
                                                   side="right"))
        vaug_pool = ctx.enter_context(tc.tile_pool(name="vaugp", bufs=1,
                                                   side="right"))
        y_pool = ctx.enter_context(tc.tile_pool(name="y", bufs=1))

        with tc.tile_pool(name="weights", bufs=1) as wpool, \
             tc.tile_pool(name="att", bufs=1) as att, \
             tc.tile_pool(name="wvp", bufs=1) as wv_pool:

            strips = {}

            def emit_strip_dmas(p, eng=None):
                """Per-pair w_q/w_k column strips [128, 2, EC, 128], one
                batched DMA per q/k."""
                e = eng or nc.sync
                st = wpool.tile([P, 2, EC, P], BF, tag="strip", bufs=3,
                                name=f"st{p}")
                for qk, col0 in ((0, p * P), (1, ATT + p * P)):
                    e.dma_start(
                        out=st[:, qk, :, :],
                        in_=wqkv_d[:, col0:col0 + P]
                        .rearrange("(k r) c -> r k c", r=P))
                strips[p] = st

            spool = ctx.enter_context(tc.tile_pool(name="ps_s", bufs=1,
                                                   space="PSUM"))
            qpool = ctx.enter_context(tc.tile_pool(name="ps_q", bufs=1,
                                                   space="PSUM"))
            avpool = ctx.enter_context(tc.tile_pool(name="ps_av", bufs=1,
                                                    space="PSUM"))

            # ---- PE warm-up: full-K matmuls engage the HAM un-throttle
            # while the startup DMAs land.
            def emit_warm(n):
                for _ in range(n):
                    ps_warm = spool.tile([P, N], FP, tag="s", bufs=2,
                                         name="warm")
                    nc.tensor.matmul(ps_warm[:, 0:HW], warm_sb[:, 0:P],
                                     warm_sb, start=True, stop=True)

            emit_warm(12)

            # ---- startup DMAs, priority order ------------------------
            # sync: pair-0 strips, xT chunks 3-5, wv, pair-1+ strips
            # scalar: xT chunks 0-2 + bias (then the ACT queue is exps only)
            emit_strip_dmas(0)
            xt6 = wpool.tile([P, EC, N], BF, tag="xT6", name="xT6")
            for k in range(3):
                nc.scalar.dma_start(out=xt6[:, k, :],
                                    in_=xT_d[k * P:(k + 1) * P, :])
            nc.scalar.dma_start(out=bT_sb, in_=bT_d[:])
            nc.sync.dma_start(
                out=xt6[:, 3:6, :],
                in_=xT_d[3 * P:6 * P, :].rearrange("(k r) c -> r k c", r=P))
            xT = [xt6[:, k, :] for k in range(EC)]
            wv6 = wv_pool.tile([P, EC, EMB], BF, tag="wv6", name="wv6")
            nc.sync.dma_start(
                out=wv6, in_=wqkv_d[:, 2 * ATT:]
                .rearrange("(k r) c -> r k c", r=P))
            wv_sb = [wv6[:, k, :] for k in range(EC)]
            emit_strip_dmas(1)
            emit_strip_dmas(2)

            # ---- helper emitters ---------------------------------------
            class QKTEmitter:
                """Emits a pair's 24 qkT matmuls a few per call: steady
                always-ready PE filler between the es-gated AV/S work."""

                def __init__(self, p):
                    self.p = p
                    self.jobs = [(qk, nn, k)
                                 for qk in range(2)
                                 for nn in range(2)
                                 for k in range(EC)]
                    self.idx = 0
                    self.psq = None
                    self.tiles = [None, None]

                def done(self):
                    return self.idx >= len(self.jobs)

                def step(self, n=3):
                    while n > 0 and not self.done():
                        qk, nn, k = self.jobs[self.idx]
                        which = "qk"[qk]
                        if self.tiles[qk] is None:
                            self.tiles[qk] = wpool.tile(
                                [P, N], BF, tag=f"{which}Tp", bufs=3,
                                name=f"{which}T{self.p}")
                        if self.psq is None:
                            self.psq = qpool.tile(
                                [P, HW], FP, tag="q", bufs=2,
                                name=f"ps{which}{self.p}_{nn}")
                        nc.tensor.matmul(
                            self.psq,
                            strips[self.p][:, qk, k, :],
                            xT[k][:, nn * HW:(nn + 1) * HW],
                            start=(k == 0),
                            stop=(k == EC - 1),
                        )
                        self.idx += 1
                        n -= 1
                        if k == EC - 1:
                            nc.vector.tensor_copy(
                                self.tiles[qk][:, nn * HW:(nn + 1) * HW],
                                self.psq)
                            self.psq = None
                    if self.done():
                        qkt[self.p] = self.tiles
                    return self.done()

            def emit_qkT(p):
                em = QKTEmitter(p)
                em.step(len(em.jobs))
                return qkt[p]

            def emit_v(t):
                va = vaug_pool.tile([P, H, P], BF, tag=f"vaug{t}",
                                    name=f"vaug{t}")
                nc.gpsimd.memset(va[:, :, D:P], 1.0)
                for (n0, n1) in ((0, 512), (512, 768)):
                    psv = avpool.tile([P, HW], FP, tag="av", bufs=2,
                                      name=f"psv{t}_{n0}")
                    w = n1 - n0
                    for k in range(EC):
                        nc.tensor.matmul(
                            psv[:, 0:w],
                            xT[k][:, t * P:(t + 1) * P],
                            wv_sb[k][:, n0:n1],
                            start=(k == 0),
                            stop=(k == EC - 1),
                        )
                    nc.vector.tensor_copy(
                        va[:, n0 // D:n1 // D, 0:D],
                        psv[:, 0:w].rearrange("p (h d) -> p h d", d=D),
                    )
                return va

            qkt = {}
            es_tiles = {}
            s_cursor = [0]

            def emit_S_unit(p, c, half):
                """S^T for key-chunk c, query-half `half`, BOTH heads in one
                PSUM tile (h0 -> cols 0:512 via row group 0, h1 -> cols
                512:1024 via row group 64). Sharing one tile means both
                matmuls' rotation dependency resolves together, so the
                row-group 2x packing engages every time; one exp covers
                both heads (per-exp instruction overhead is ~125ns, so
                fewer, larger exps win)."""
                qT, kT = qkt[p]
                t = spool.tile([P, N], FP, tag="s", bufs=2,
                               name=f"s{p}_{c}_{half}")
                for i, base in ((0, 0), (1, 64)):
                    nc.tensor.matmul(
                        t[:, i * HW:(i + 1) * HW],
                        kT[base:base + D, c * P:(c + 1) * P],
                        qT[base:base + D, half * HW:(half + 1) * HW],
                        start=True,
                        stop=True,
                    )
                es = att.tile([P, N], BF, tag="expS", bufs=28,
                              name=f"es{p}_{c}_{half}")
                nc.scalar.activation(
                    es, t, mybir.ActivationFunctionType.Exp, scale=SCALE)
                return es

            def pump_S(limit):
                while s_cursor[0] < min(limit, NP * NT * 2):
                    u = s_cursor[0]
                    p_, w = divmod(u, 2 * NT)
                    half_, c_ = divmod(w, NT)
                    if p_ not in qkt:
                        break
                    es_tiles[(p_, c_, half_)] = emit_S_unit(p_, c_, half_)
                    s_cursor[0] += 1

            def emit_normalize(p, i, half, av_t):
                # Plain-op Newton reciprocal: seed z0 = bitcast(~x)*c0 = -y0,
                # one NR step z1 = (x*z0 + 2)*z0 = -y1, final multiply folds
                # the sign: outT = (raw * -1) * z1 = raw * y1.
                # (custom DVE ops like reciprocal_approx_fast run in CoreSim
                # but produce garbage on this runtime - plain ops only)
                x = av_t[D:2 * D, :]               # [64,512] denom (repl)
                nx = att.tile([D, HW], FP, tag="nrm", bufs=4,
                              name=f"nx{p}_{i}_{half}")
                nc.vector.tensor_scalar(
                    out=nx.bitcast(mybir.dt.int32),
                    in0=x.bitcast(mybir.dt.int32),
                    scalar1=-1, scalar2=None,
                    op0=mybir.AluOpType.bitwise_xor)
                z0 = att.tile([D, HW], FP, tag="nrm", bufs=4,
                              name=f"z0{p}_{i}_{half}")
                nc.vector.tensor_scalar_mul(z0, nx, 0.23549792)
                pr = att.tile([D, HW], FP, tag="nrm", bufs=4,
                              name=f"pr{p}_{i}_{half}")
                nc.vector.tensor_mul(pr, x, z0)
                z1 = att.tile([D, HW], FP, tag="nrm", bufs=4,
                              name=f"z1{p}_{i}_{half}")
                nc.vector.scalar_tensor_tensor(
                    out=z1, in0=pr, scalar=-2.0, in1=z0,
                    op0=mybir.AluOpType.subtract,
                    op1=mybir.AluOpType.mult)
                nc.vector.scalar_tensor_tensor(
                    out=outT[p][i * D:(i + 1) * D,
                                half * HW:(half + 1) * HW],
                    in0=av_t[0:D, :], scalar=-1.0, in1=z1,
                    op0=mybir.AluOpType.mult,
                    op1=mybir.AluOpType.mult)

            # ---- y output projection helpers ---------------------------
            y_live = {}

            def emit_y(e, nn, ks, finish=False, drain="vector", psum=None):
                t = y_live.get((e, nn))
                if t is None:
                    if psum is None:
                        psum = qpool.tile([P, HW], FP, tag="q", bufs=2,
                                          name=f"psy{e}_{nn}")
                    t = psum
                    y_live[(e, nn)] = t
                for k in ks:
                    nc.tensor.matmul(
                        t,
                        wout_sb[k][:, e * P:(e + 1) * P],
                        outT[k][:, nn * HW:(nn + 1) * HW],
                        start=(k == 0),
                        stop=(finish and k == NP - 1),
                    )
                if finish:
                    y_sb = y_pool.tile([P, HW], FP, tag="y", bufs=4,
                                       name=f"ysb{e}_{nn}")
                    if drain == "vector":
                        # DVE is idle here; +bias via per-partition scalar
                        nc.vector.tensor_scalar_add(
                            y_sb, t, bT_sb[:, e:e + 1])
                    else:
                        # ACT is idle post-exp; Identity allows AP bias
                        nc.scalar.activation(
                            y_sb, t, mybir.ActivationFunctionType.Identity,
                            bias=bT_sb[:, e:e + 1])
                    dma_eng = nc.sync if e % 2 == 0 else nc.scalar
                    dma_eng.dma_start(
                        out=y_d[e * P:(e + 1) * P, nn * HW:(nn + 1) * HW],
                        in_=y_sb)
                    del y_live[(e, nn)]

            # ---- prologue: qkT(0) (warm-MM filler absorbs the DMA
            # arrival jitter), pump, qkT(1), then S+v interleave -------
            qkt0 = QKTEmitter(0)
            while not qkt0.done():
                qkt0.step(6)
                emit_warm(3)
            pump_S(2)
            qkt1 = QKTEmitter(1)
            for _ in range(3):
                qkt1.step(8)
                pump_S(s_cursor[0] + 1)
            vaug = []
            for c in range(NT):
                pump_S(2 * (c + 1) + 3)
                vaug.append(emit_v(c))
                if c == 0:
                    emit_strip_dmas(3)
                if c == 4:
                    emit_strip_dmas(4)

            # w_out load rides the sync queue here (arrives mid-attention)
            wo6 = wout_pool.tile([P, EC, EMB], BF, tag="wout6", name="wout6")
            nc.sync.dma_start(
                out=wo6, in_=wout_d.rearrange("(k r) c -> r k c", r=P))
            wout_sb = [wo6[:, k, :] for k in range(EC)]

            # ---- main pair loop (AV per query-half) --------------------
            for p in range(NP):
                qkt_next = QKTEmitter(p + 2) if p + 2 < NP else None
                for half in range(2):
                    av_ts = [
                        avpool.tile([P, HW], FP, tag="av", bufs=2,
                                    name=f"av{p}_{i}_{half}")
                        for i in range(2)
                    ]
                    for c in range(NT):
                        # The PE queue is strict FIFO: put stall-prone S
                        # matmuls AFTER always-ready AV/qkT work, except at
                        # the half boundary (c==0) where AV itself waits on
                        # the av-psum rotation and S is the filler.
                        if c == 0:
                            pump_S((p * 2 + half) * NT + c + 2 * NT + 6)
                        for i in range(2):
                            nc.tensor.matmul(
                                av_ts[i],
                                vaug[c][:, 2 * p + i, :],
                                es_tiles[(p, c, half)][:,
                                                       i * HW:(i + 1) * HW],
                                start=(c == 0),
                                stop=(c == NT - 1),
                            )
                        if qkt_next is not None:
                            qkt_next.step(2 if half == 0 else 1)
                        if half == 0 and c == 4 and p == 0 and NP > 5:
                            emit_strip_dmas(5)
                        if p == NP - 1:
                            # weave the output projection into pair 5:
                            # k<=4 partials are dependency-free filler; the
                            # k=5 finishers wait only normalize(5, half=0)
                            # which completes early in half 1.
                            if half == 0 and c == 2:
                                emit_y(0, 0, range(NP - 1))
                            if half == 0 and c == 5:
                                emit_y(1, 0, range(NP - 1))
                            if half == 1 and c == 4:
                                emit_y(0, 0, [NP - 1], finish=True)
                            if half == 1 and c == 5:
                                emit_y(1, 0, [NP - 1], finish=True)
                            if half == 1 and c == 6:
                                emit_y(2, 0, range(NP), finish=True,
                                       drain="scalar")
                            if half == 1 and c == 7:
                                emit_y(3, 0, range(NP), finish=True,
                                       drain="scalar")
                        if c > 0:
                            pump_S((p * 2 + half) * NT + c + 2 * NT + 6)
                    if qkt_next is not None and half == 0:
                        qkt_next.step(4)
                    emit_normalize(p, 0, half, av_ts[0])
                    emit_normalize(p, 1, half, av_ts[1])
                    # keep the HAM activity monitor fed across the
                    # normalize-chain stall (LDWEIGHTS needs no PSUM bank)
                    for _ in range(3):
                        nc.tensor.ldweights(weights=warm_sb[:, 0:P])

        # ---- tail: the S-pool tiles are drained by now; use them as
        # extra y accumulators (one [128,N] tile hosts two y halves) so
        # all remaining output-projection matmuls run back-to-back.
        # nn=0 work (only needs normalize(5,0), long done) goes first
        # and covers the normalize(5,1) DVE chains; nn=1 k=5 last.
        sA = spool.tile([P, N], FP, tag="s", bufs=2, name="ytail_a")
        emit_y(4, 0, range(NP), finish=True, drain="scalar",
               psum=sA[:, 0:HW])
        emit_y(5, 0, range(NP), finish=True, drain="scalar",
               psum=sA[:, HW:N])
        sB = spool.tile([P, N], FP, tag="s", bufs=2, name="ytail_b")
        emit_y(0, 1, range(NP - 1), psum=sB[:, 0:HW])
        emit_y(1, 1, range(NP - 1), psum=sB[:, HW:N])
        sC = spool.tile([P, N], FP, tag="s", bufs=2, name="ytail_c")
        emit_y(4, 1, range(NP - 1), psum=sC[:, 0:HW])
        emit_y(5, 1, range(NP - 1), psum=sC[:, HW:N])
        emit_y(2, 1, range(NP - 1))
        emit_y(3, 1, range(NP - 1))
        # k=5 finishers: drains alternate ACT/DVE so the last few pipeline
        emit_y(0, 1, [NP - 1], finish=True, drain="scalar")
        emit_y(1, 1, [NP - 1], finish=True, drain="vector")
        emit_y(4, 1, [NP - 1], finish=True, drain="scalar")
        emit_y(5, 1, [NP - 1], finish=True, drain="vector")
        emit_y(2, 1, [NP - 1], finish=True, drain="scalar")
        emit_y(3, 1, [NP - 1], finish=True, drain="vector")


_NC_CACHE = None


def _build_nc(reps=1):
    global _NC_CACHE
    if reps == 1 and _NC_CACHE is not None:
        return _NC_CACHE
    nc = bacc.Bacc("TRN2", target_bir_lowering=False, debug=False,
                   num_devices=N_CORES)
    xT_d = nc.declare_dram_parameter("xT", [EMB, N], BF, isOutput=False)
    wqkv_d = nc.declare_dram_parameter("w_qkv", [EMB, 3 * ATT], BF, isOutput=False)
    wout_d = nc.declare_dram_parameter("w_out", [ATT, EMB], BF, isOutput=False)
    bT_d = nc.declare_dram_parameter("bT", [P, EC], FP, isOutput=False)
    y_d = nc.declare_dram_parameter("y", [EMB, N], FP, isOutput=True)
    with tile.TileContext(nc) as tc:
        for _ in range(reps):
            _emit_kernel(tc, xT_d, wqkv_d, wout_d, bT_d, y_d)
    nc.compile()
    if reps == 1:
        _NC_CACHE = nc
    return nc


def make_in_maps(x, w_qkv, w_out, b_out):
    BFnp = ml_dtypes.bfloat16
    x = np.asarray(x, dtype=np.float32)
    w_qkv = np.asarray(w_qkv, dtype=np.float32).astype(BFnp)
    w_out = np.asarray(w_out, dtype=np.float32).astype(BFnp)
    bT = np.ascontiguousarray(
        np.asarray(b_out, dtype=np.float32).reshape(EC, P).T)
    assert x.shape == (B, N, EMB)
    xT = [np.ascontiguousarray(x[i].T).astype(BFnp) for i in range(B)]
    return [
        {"xT": xT[i], "w_qkv": w_qkv, "w_out": w_out, "bT": bT}
        for i in range(N_CORES)
    ]


def postprocess_core_output(y):
    return np.ascontiguousarray(np.asarray(y).T)


def run_sharded(x, w_qkv, w_out, b_out, **run_kwargs):
    """Shard over batch, run on 8 cores, gather. Returns (out, BassKernelResults)."""
    nc = _build_nc()
    in_maps = make_in_maps(x, w_qkv, w_out, b_out)
    res = run_bass_kernel_spmd(nc, in_maps, core_ids=list(range(N_CORES)),
                               **run_kwargs)
    out = np.stack(
        [postprocess_core_output(res.results[i]["y"]) for i in range(N_CORES)],
        axis=0)
    return out, res


def kernel(x, w_qkv, w_out, b_out):
    out, _ = run_sharded(x, w_qkv, w_out, b_out)
    return out
